# revision 1
# baseline (speedup 1.0000x reference)
"""Single-head attention (B=4, T=4096, D=1024, H=64) on 8 TRN2 NeuronCores.

Sharding: data-parallel over B (4 batches x 2 cores); within a batch each
core owns 2048 q rows and streams the batch's full kv set.

Device kernel (bf16 compute, f32 softmax accumulation):
  - kv compaction: the host knows the padding mask, and attention is
    permutation-invariant over kv positions, so each core receives only the
    batch's unmasked kv rows (first, in order) padded with masked filler to
    NKV=2176; filler is killed by the exp bias. This roughly halves the
    attention/exp work vs processing all 4096 positions.
  - x arrives bf16 pre-split: xq [2048, D] (the core's q rows) and
    xkv [NKV, D] (compacted batch kv rows). DMA-transposes land xqT/xkvT
    directly in SBUF (sync HWDGE ring only; the scalar ring corrupts).
  - Projections: q alone (M=64); k|v packed into one 128-col stationary.
    v gets a ones column appended (softmax denominator via the PV matmul).
  - Attention, tbp-major (t-block pairs sequentially, so the first half's
    finalize overlaps the second half's attention and only two [65,512]
    accumulators are live -> 3 deep QK psum pipeline): per s-chunk,
    QK matmuls at stage k, 1024-wide ACT exp(psum*scale + mask_bias) at
    k-1, PV accumulate at k-2.
  - Finalize: PE-transpose [65,...] back to [t, 65], divide by the prob
    row-sum, add bv, single output DMA.
"""
import numpy as np
import ml_dtypes

import concourse.bass as bass
import concourse.mybir as mybir
from concourse import bacc
from concourse.tile import TileContext
from concourse.masks import make_identity
from concourse.bass_utils import run_bass_kernel_spmd

B, T, D, H = 4, 4096, 1024, 64
N_CORES = 8
TQ = T // 2            # q rows per core
QB = TQ // 512         # q 512-col blocks
DC = D // 128          # contraction chunks
NKV = 2176             # compacted kv positions (binomial 2048+-32, +4 sigma)
SCK = NKV // 128       # kv chunks of 128
SHALVES = (1152, 1024)  # kv DMA s-half sizes (128-multiples)
SCALE = float(H) ** -0.5

F32 = mybir.dt.float32
BF16 = mybir.dt.bfloat16

# kv projection blocks (within each s-half): 128-multiples tiling each half
KV_BLOCKS = [(0, 512), (512, 512), (1024, 128), (1152, 512), (1664, 512)]


def build_kernel():
    nc = bacc.Bacc()
    xq = nc.dram_tensor("xq", [TQ, D], BF16, kind="ExternalInput")
    xkv = nc.dram_tensor("xkv", [NKV, D], BF16, kind="ExternalInput")
    wt = nc.dram_tensor("wt", [D, 3 * H], BF16, kind="ExternalInput")  # [wqT|wkT|wvT]
    qkb = nc.dram_tensor("qkb", [128, 2], F32, kind="ExternalInput")   # c0 bq, c1 bk
    bv128 = nc.dram_tensor("bv128", [128, H], F32, kind="ExternalInput")
    maskb = nc.dram_tensor("maskb", [128, SCK], F32, kind="ExternalInput")
    out = nc.dram_tensor("out", [TQ, H], F32, kind="ExternalOutput")

    with TileContext(nc) as tc:
        with tc.tile_pool(name="const", bufs=1) as const:
            xqT = const.tile([128, DC, TQ], BF16)
            xkvT = const.tile([128, DC, NKV], BF16)
            for dc in range(DC):
                nc.sync.dma_start_transpose(
                    xqT[:, dc, :], xq[:, dc * 128:(dc + 1) * 128])
            off = 0
            for shs in SHALVES:
                ssl = slice(off, off + shs)
                off += shs
                for dc in range(DC):
                    nc.sync.dma_start_transpose(
                        xkvT[:, dc, ssl], xkv[ssl, dc * 128:(dc + 1) * 128])

            wt_sb = const.tile([128, DC, 3 * H], BF16)
            nc.gpsimd.dma_start(
                out=wt_sb, in_=wt.rearrange("(c p) w -> p c w", p=128))
            qkb_sb = const.tile([128, 2], F32)
            nc.gpsimd.dma_start(out=qkb_sb, in_=qkb[:, :])
            bv_sb = const.tile([128, H], F32)
            nc.gpsimd.dma_start(out=bv_sb, in_=bv128[:, :])
            maskb_sb = const.tile([128, SCK], F32)
            nc.gpsimd.dma_start(out=maskb_sb, in_=maskb[:, :])
            ident32 = const.tile([128, 128], F32)
            make_identity(nc, ident32)
            identb = const.tile([128, 128], BF16)
            nc.vector.tensor_copy(identb, ident32)

            qT_sb = const.tile([H, TQ], BF16)
            kT_sb = const.tile([H, NKV], BF16)
            v_sb = const.tile([128, SCK, H + 1], BF16)
            out_sb = const.tile([128, TQ // 128, H], F32)

            # ---------------- Phase A: projections ----------------
            with tc.tile_pool(name="vstage", bufs=2) as vstage, \
                 tc.tile_pool(name="psq", bufs=2, space="PSUM") as psqp, \
                 tc.tile_pool(name="pskv", bufs=2, space="PSUM") as pskvp, \
                 tc.tile_pool(name="psvt", bufs=2, space="PSUM") as psvtp:
                # q projections (M=64)
                for tb in range(QB):
                    tsl = slice(tb * 512, (tb + 1) * 512)
                    ps_q = psqp.tile([H, 512], F32, tag="psq")
                    for dc in range(DC):
                        nc.tensor.matmul(
                            ps_q, wt_sb[:, dc, 0:H], xqT[:, dc, tsl],
                            start=(dc == 0), stop=(dc == DC - 1))
                    nc.scalar.activation(
                        qT_sb[:, tsl], ps_q,
                        mybir.ActivationFunctionType.Identity,
                        bias=qkb_sb[0:H, 0:1], scale=1.0)

                # k|v projections (M=128: rows 0-63 k, 64-127 v)
                for off, sz in KV_BLOCKS:
                    ssl = slice(off, off + sz)
                    ps_kv = pskvp.tile([128, 512], F32, tag="pskv")
                    for dc in range(DC):
                        nc.tensor.matmul(
                            ps_kv[:, 0:sz], wt_sb[:, dc, H:H + 128],
                            xkvT[:, dc, ssl],
                            start=(dc == 0), stop=(dc == DC - 1))
                    nc.scalar.activation(
                        kT_sb[:, ssl], ps_kv[0:H, 0:sz],
                        mybir.ActivationFunctionType.Identity,
                        bias=qkb_sb[0:H, 1:2], scale=1.0)
                    vt_ext = vstage.tile([H + 1, 512], BF16)
                    nc.scalar.copy(vt_ext[0:H, 0:sz], ps_kv[H:128, 0:sz])
                    nc.vector.memset(vt_ext[H:H + 1, 0:sz], 1.0)
                    nsub = sz // 128
                    psvt = psvtp.tile([128, 4, H + 2], BF16, tag="psvt")
                    for j in range(nsub):
                        nc.tensor.transpose(
                            psvt[:, j, 0:H + 1],
                            vt_ext[:, j * 128:(j + 1) * 128],
                            identb[0:H + 1, 0:H + 1])
                    nc.vector.tensor_copy(
                        v_sb[:, off // 128:off // 128 + nsub, :],
                        psvt[:, 0:nsub, 0:H + 1])

            # ---------------- Phase B: attention ----------------
            # Pipeline over pairs (sc, tbp): QK at stage k, exp at k-1,
            # PV at k-2 -> PE and ACT run concurrently.
            with tc.tile_pool(name="ptile", bufs=3) as ptile, \
                 tc.tile_pool(name="po", bufs=1, space="PSUM") as po, \
                 tc.tile_pool(name="pqk", bufs=3, space="PSUM") as pqk, \
                 tc.tile_pool(name="ostage", bufs=2) as ostage, \
                 tc.tile_pool(name="rec", bufs=4) as recp:
                qk_tiles = {}
                p_tiles = {}
                ps_o = [None] * QB

                def emit_qk(sc, tbp):
                    ps_qk = pqk.tile([128, 1024], F32, tag="ps_qk",
                                     name=f"ps_qk{sc % 3}")
                    for j in range(2):
                        tb = 2 * tbp + j
                        nc.tensor.matmul(
                            ps_qk[:, j * 512:(j + 1) * 512],
                            kT_sb[:, sc * 128:(sc + 1) * 128],
                            qT_sb[:, tb * 512:(tb + 1) * 512],
                            start=True, stop=True)
                    qk_tiles[sc] = ps_qk

                def emit_exp(sc):
                    p = ptile.tile([128, 1024], BF16)
                    nc.scalar.activation(
                        p, qk_tiles.pop(sc), mybir.ActivationFunctionType.Exp,
                        bias=maskb_sb[:, sc:sc + 1], scale=SCALE)
                    p_tiles[sc] = p

                def emit_pv(sc, tbp):
                    p = p_tiles.pop(sc)
                    for j in range(2):
                        tb = 2 * tbp + j
                        nc.tensor.matmul(
                            ps_o[tb], v_sb[:, sc, :],
                            p[:, j * 512:(j + 1) * 512],
                            start=(sc == 0), stop=(sc == SCK - 1))

                def finalize_tb(tb):
                    # fin transposes borrow the (drained) ps_o slot of this tb
                    o_sb = ostage.tile([H + 1, 512], F32)
                    nc.any.tensor_copy(o_sb, ps_o[tb])
                    for j in range(4):
                        ps_ot = po.tile([128, H + 1], F32, tag=f"ps_o{tb % 2}",
                                        name=f"ps_ot{tb}_{j}")
                        nc.tensor.transpose(
                            ps_ot,
                            o_sb[:, j * 128:(j + 1) * 128],
                            ident32[0:H + 1, 0:H + 1])
                        rec = recp.tile([128, 1], F32)
                        nc.vector.reciprocal(rec, ps_ot[:, H:H + 1])
                        oc = out_sb[:, 4 * tb + j, :]
                        nc.vector.tensor_scalar_mul(oc, ps_ot[:, 0:H], rec)
                        nc.vector.tensor_add(oc, oc, bv_sb)

                # tbp-major: half 0's finalize overlaps half 1's attention;
                # only 2 accumulators live per half -> po holds 2 banks and
                # pqk gets a third slot (deeper QK pipelining).
                for tbp in range(QB // 2):
                    for tb in (2 * tbp, 2 * tbp + 1):
                        ps_o[tb] = po.tile([H + 1, 512], F32, tag=f"ps_o{tb % 2}",
                                           name=f"ps_o{tb}")
                    for k in range(SCK + 2):
                        if k >= 2:
                            emit_pv(k - 2, tbp)
                        if 1 <= k < SCK + 1:
                            emit_exp(k - 1)
                        if k < SCK:
                            emit_qk(k, tbp)
                    finalize_tb(2 * tbp)
                    finalize_tb(2 * tbp + 1)

            nc.sync.dma_start(
                out=out.rearrange("(i p) h -> p i h", p=128), in_=out_sb)

    nc.finalize()
    return nc


_NC_CACHE = None


def _get_nc():
    global _NC_CACHE
    if _NC_CACHE is None:
        _NC_CACHE = build_kernel()
    return _NC_CACHE


def make_in_maps(x, mask, wq, bq, wk, bk, wv, bv):
    x = np.asarray(x, dtype=np.float32)
    mask = np.asarray(mask)
    wt = np.concatenate(
        [np.asarray(wq, np.float32).T, np.asarray(wk, np.float32).T,
         np.asarray(wv, np.float32).T], axis=1).astype(ml_dtypes.bfloat16)
    bqf = np.asarray(bq, np.float32)
    bkf = np.asarray(bk, np.float32)
    zf = np.zeros(H, np.float32)
    qkb = np.stack([np.concatenate([bqf, zf]),
                    np.concatenate([bkf, zf])], axis=1).copy()
    bv128 = np.tile(np.asarray(bv, np.float32)[None, :], (128, 1)).copy()

    in_maps = []
    per_batch = {}
    for b in range(B):
        mb = mask[b].astype(bool)
        keep = np.flatnonzero(mb)
        fill = np.flatnonzero(~mb)
        cnt = len(keep)
        assert cnt <= NKV, f"unmasked kv count {cnt} exceeds NKV={NKV}"
        order = np.concatenate([keep, fill])[:NKV]
        xkv = np.ascontiguousarray(x[b][order]).astype(ml_dtypes.bfloat16)
        biasvals = np.where(np.arange(NKV) < cnt, 0.0, -1e9).astype(np.float32)
        maskb = np.ascontiguousarray(
            biasvals.reshape(SCK, 128).T).copy()
        per_batch[b] = (xkv, maskb)

    for c in range(N_CORES):
        b, half = c // 2, c % 2
        xkv, maskb = per_batch[b]
        xqb = np.ascontiguousarray(
            x[b, half * TQ:(half + 1) * TQ]).astype(ml_dtypes.bfloat16)
        in_maps.append({
            "xq": xqb,
            "xkv": xkv,
            "wt": wt,
            "qkb": qkb,
            "bv128": bv128,
            "maskb": maskb,
        })
    return in_maps


def run(in_maps, **kwargs):
    nc = _get_nc()
    return run_bass_kernel_spmd(nc, in_maps, core_ids=list(range(N_CORES)), **kwargs)


def kernel(x, mask, wq, bq, wk, bk, wv, bv):
    in_maps = make_in_maps(x, mask, wq, bq, wk, bk, wv, bv)
    res = run(in_maps)
    out = np.empty((B, T, H), dtype=np.float32)
    for c in range(N_CORES):
        b, half = c // 2, c % 2
        out[b, half * TQ:(half + 1) * TQ] = res.results[c]["out"]
    return out



# revision 2
# speedup vs baseline: 1.8879x; 1.8879x over previous
"""Single-head attention (B=4, T=4096, D=1024, H=64) on 8 TRN2 NeuronCores.

Sharding: data-parallel over B (4 batches x 2 cores); within a batch each
core owns 2048 q rows and streams the batch's full kv set.

v2 design (bf16 compute, f32 softmax accumulation):
  - All transposes happen on the host: xq/xkv/w arrive pre-transposed and
    pre-laid-out so every input DMA is a plain contiguous HWDGE load.
  - kv compaction: unmasked kv rows first; filler rows are set to X where
    X @ wv.T = -bv, so after the on-device bias add the filler v rows are
    exactly zero. The softmax denominator comes from a ones/mask row
    appended to v (masked during the v-transpose copy), so the exp needs
    no mask bias at all and filler kv rows contribute exactly nothing.
  - q projection: stationary holds [wq | wq] (M=128), so the PE emits qT
    duplicated across both partition halves at no extra cost - needed for
    row-tiled QK.
  - kv projection: stationary [wv | wk] (M=128). kT lands split-half:
    even kv chunks on partitions 0-63, odd chunks on 64-127.
  - QK is row-tiled: contraction K=H=64 only fills half the PE array, so
    two kv chunks run concurrently on the top/bottom array halves
    (tile_position (0,0) and (64,0)), doubling QK throughput.
  - Softmax: ACT does exp only (scale immediate, no bias); every other
    elementwise op (bias adds, copies, finalize) runs on DVE.
  - PV: v|mask stationary [128, 65]; denominator accumulates as row 64.
  - Finalize: PE-transpose [65, 128] blocks, DVE reciprocal + scale.
"""
import numpy as np
import ml_dtypes

import concourse.bass as bass
import concourse.mybir as mybir
from concourse import bacc
from concourse.tile import TileContext
from concourse.masks import make_identity
from concourse.bass_utils import run_bass_kernel_spmd

B, T, D, H = 4, 4096, 1024, 64
N_CORES = 8
TQ = T // 2            # q rows per core
QB = TQ // 512         # q 512-col blocks
DC = D // 128          # contraction chunks
NKV = 2176             # compacted kv positions (max count 2076 rounded up)
SCK = NKV // 128       # kv chunks of 128 (17)
NPAIR = SCK // 2       # row-tiled chunk pairs (8) + 1 tail chunk
SCALE = float(H) ** -0.5

F32 = mybir.dt.float32
BF16 = mybir.dt.bfloat16

# kv projection blocks: 4x512 + 1x128 tiling NKV
KV_BLOCKS = [(0, 512), (512, 512), (1024, 512), (1536, 512), (2048, 128)]


def build_kernel():
    nc = bacc.Bacc()
    # pre-transposed/pre-laid-out inputs (see make_in_maps)
    xqT = nc.dram_tensor("xqT", [128, QB, DC, 512], BF16, kind="ExternalInput")
    xkvT = nc.dram_tensor("xkvT", [128, DC, NKV], BF16, kind="ExternalInput")
    wt = nc.dram_tensor("wt", [128, DC, 4 * H], BF16, kind="ExternalInput")
    bq128 = nc.dram_tensor("bq128", [128, 1], F32, kind="ExternalInput")
    bkv = nc.dram_tensor("bkv", [128, 2], F32, kind="ExternalInput")
    maskc = nc.dram_tensor("maskc", [128, SCK], F32, kind="ExternalInput")
    out = nc.dram_tensor("out", [128, TQ // 128, H], F32, kind="ExternalOutput")

    with TileContext(nc) as tc:
        with tc.tile_pool(name="const", bufs=1) as const:
            xqT_sb = const.tile([128, QB, DC, 512], BF16)
            xkvT_sb = const.tile([128, DC, NKV], BF16)
            nc.sync.dma_start(out=xqT_sb[:, 0], in_=xqT[:, 0])
            for off, sz in KV_BLOCKS:
                nc.sync.dma_start(
                    out=xkvT_sb[:, :, off:off + sz], in_=xkvT[:, :, off:off + sz])
            for tb in range(1, QB):
                nc.sync.dma_start(out=xqT_sb[:, tb], in_=xqT[:, tb])

            wt_sb = const.tile([128, DC, 4 * H], BF16)
            nc.gpsimd.dma_start(out=wt_sb, in_=wt[:, :, :])
            bq_sb = const.tile([128, 1], F32)
            nc.gpsimd.dma_start(out=bq_sb, in_=bq128[:, :])
            bkv_sb = const.tile([128, 2], F32)
            nc.gpsimd.dma_start(out=bkv_sb, in_=bkv[:, :])
            mask_sb = const.tile([128, SCK], F32)
            nc.gpsimd.dma_start(out=mask_sb, in_=maskc[:, :])
            ident32 = const.tile([128, 128], F32)
            make_identity(nc, ident32)
            identb = const.tile([128, 128], BF16)
            nc.vector.tensor_copy(identb, ident32)

            qT2 = const.tile([128, TQ], BF16)        # rows 0-63 qT, 64-127 dup
            kT2 = const.tile([128, (NPAIR + 1) * 128], BF16)  # even|odd halves
            v_sb = const.tile([128, SCK, H + 1], BF16)
            out_sb = const.tile([128, TQ // 128, H], F32)

            # ---------------- Phase A: projections ----------------
            with tc.tile_pool(name="vstage", bufs=2) as vstage, \
                 tc.tile_pool(name="psq", bufs=2, space="PSUM") as psqp, \
                 tc.tile_pool(name="pskv", bufs=2, space="PSUM") as pskvp, \
                 tc.tile_pool(name="psvt", bufs=2, space="PSUM") as psvtp:

                def emit_qproj(tb):
                    tsl = slice(tb * 512, (tb + 1) * 512)
                    ps_q = psqp.tile([128, 512], F32, tag="psq")
                    for dc in range(DC):
                        nc.tensor.matmul(
                            ps_q, wt_sb[:, dc, 0:128], xqT_sb[:, tb, dc, :],
                            start=(dc == 0), stop=(dc == DC - 1))
                    nc.vector.tensor_scalar_add(qT2[:, tsl], ps_q, bq_sb)

                def emit_kvproj(off, sz):
                    ssl = slice(off, off + sz)
                    ps_kv = pskvp.tile([128, 512], F32, tag="pskv")
                    for dc in range(DC):
                        nc.tensor.matmul(
                            ps_kv[:, 0:sz], wt_sb[:, dc, 128:256],
                            xkvT_sb[:, dc, ssl],
                            start=(dc == 0), stop=(dc == DC - 1))
                    # k rows (psum 64-127) -> kT2 split halves + bk
                    for j in range(sz // 128):
                        c = off // 128 + j
                        half, pos = c % 2, (c // 2) * 128
                        nc.vector.tensor_scalar_add(
                            kT2[64 * half:64 * half + 64, pos:pos + 128],
                            ps_kv[64:128, j * 128:(j + 1) * 128],
                            bkv_sb[64:128, 1:2])
                    # v rows (psum 0-63) + bv -> vt_ext; row 64 = ones
                    vt = vstage.tile([H + 1, 512], BF16)
                    nc.vector.tensor_scalar_add(
                        vt[0:H, 0:sz], ps_kv[0:H, 0:sz], bkv_sb[0:H, 0:1])
                    nc.vector.memset(vt[H:H + 1, 0:sz], 1.0)
                    psv = psvtp.tile([128, 4, H + 2], BF16, tag="psvt")
                    for j in range(sz // 128):
                        nc.tensor.transpose(
                            psv[:, j, 0:H + 1],
                            vt[:, j * 128:(j + 1) * 128],
                            identb[0:H + 1, 0:H + 1])
                    for j in range(sz // 128):
                        c = off // 128 + j
                        nc.vector.tensor_scalar_mul(
                            v_sb[:, c, :], psv[:, j, 0:H + 1],
                            mask_sb[:, c:c + 1])

                emit_qproj(0)
                for off, sz in KV_BLOCKS:
                    emit_kvproj(off, sz)
                for tb in range(1, QB):
                    emit_qproj(tb)

            # ---------------- Phase B: attention ----------------
            # Per tb (512 q cols): pipeline over chunk-pair steps k:
            # QK (row-tiled pair) at stage k, exp at k-1, PV at k-2.
            with tc.tile_pool(name="ptile", bufs=3) as ptile, \
                 tc.tile_pool(name="po", bufs=1, space="PSUM") as po, \
                 tc.tile_pool(name="pqk", bufs=3, space="PSUM") as pqk, \
                 tc.tile_pool(name="ostage", bufs=2) as ostage, \
                 tc.tile_pool(name="rec", bufs=4) as recp:
                NSTEP = NPAIR + 1   # 8 pairs + tail chunk 16
                qk_tiles = {}
                p_tiles = {}
                ps_o = [None] * QB

                def emit_qk(tb, k):
                    tsl = slice(tb * 512, (tb + 1) * 512)
                    ps = pqk.tile([128, 1024], F32, tag="ps_qk",
                                  name=f"ps_qk{(tb * NSTEP + k) % 3}")
                    ksl = slice(k * 128, (k + 1) * 128)
                    nc.tensor.matmul(
                        ps[:, 0:512], kT2[0:64, ksl], qT2[0:64, tsl],
                        start=True, stop=True)
                    if k < NPAIR:
                        nc.tensor.matmul(
                            ps[:, 512:1024], kT2[64:128, ksl], qT2[64:128, tsl],
                            start=True, stop=True)
                    qk_tiles[k] = ps

                def emit_exp(k):
                    n = 1024 if k < NPAIR else 512
                    p = ptile.tile([128, 1024], BF16)
                    nc.scalar.activation(
                        p[:, 0:n], qk_tiles.pop(k)[:, 0:n],
                        mybir.ActivationFunctionType.Exp, scale=SCALE)
                    p_tiles[k] = p

                def emit_pv(tb, k):
                    p = p_tiles.pop(k)
                    nc.tensor.matmul(
                        ps_o[tb], v_sb[:, 2 * k, :], p[:, 0:512],
                        start=(k == 0), stop=(k == NSTEP - 1))
                    if k < NPAIR:
                        nc.tensor.matmul(
                            ps_o[tb], v_sb[:, 2 * k + 1, :], p[:, 512:1024],
                            start=False, stop=False)

                def finalize_tb(tb):
                    o_sb = ostage.tile([H + 1, 512], F32)
                    nc.vector.tensor_copy(o_sb, ps_o[tb])
                    for j in range(4):
                        ps_ot = po.tile([128, H + 1], F32, tag=f"ps_o{tb % 2}",
                                        name=f"ps_ot{tb}_{j}")
                        nc.tensor.transpose(
                            ps_ot,
                            o_sb[:, j * 128:(j + 1) * 128],
                            ident32[0:H + 1, 0:H + 1])
                        rec = recp.tile([128, 1], F32)
                        nc.vector.reciprocal(rec, ps_ot[:, H:H + 1])
                        nc.vector.tensor_scalar_mul(
                            out_sb[:, 4 * tb + j, :], ps_ot[:, 0:H], rec)
                    nc.sync.dma_start(
                        out=out[:, 4 * tb:4 * tb + 4, :],
                        in_=out_sb[:, 4 * tb:4 * tb + 4, :])

                for tb in range(QB):
                    ps_o[tb] = po.tile([H + 1, 512], F32, tag=f"ps_o{tb % 2}",
                                       name=f"ps_o{tb}")
                    for k in range(NSTEP + 2):
                        if k >= 2:
                            emit_pv(tb, k - 2)
                        if 1 <= k < NSTEP + 1:
                            emit_exp(k - 1)
                        if k < NSTEP:
                            emit_qk(tb, k)
                    finalize_tb(tb)

    nc.finalize()
    return nc


_NC_CACHE = None


def _get_nc():
    global _NC_CACHE
    if _NC_CACHE is None:
        _NC_CACHE = build_kernel()
    return _NC_CACHE


def make_in_maps(x, mask, wq, bq, wk, bk, wv, bv):
    x = np.asarray(x, dtype=np.float32)
    mask = np.asarray(mask)
    wqf = np.asarray(wq, np.float32)
    wkf = np.asarray(wk, np.float32)
    wvf = np.asarray(wv, np.float32)
    bqf = np.asarray(bq, np.float32)
    bkf = np.asarray(bk, np.float32)
    bvf = np.asarray(bv, np.float32)

    # stationary columns: [wq | wq | wv | wk]  (q duplicated for row-tiled QK)
    wt_full = np.concatenate(
        [wqf.T, wqf.T, wvf.T, wkf.T], axis=1)          # [D, 4H]
    wt = np.ascontiguousarray(
        wt_full.reshape(DC, 128, 4 * H).transpose(1, 0, 2)
    ).astype(ml_dtypes.bfloat16)                        # [128, DC, 4H]

    bq128 = np.concatenate([bqf, bqf])[:, None].astype(np.float32)  # [128,1]
    bkv = np.zeros((128, 2), np.float32)
    bkv[0:H, 0] = bvf
    bkv[H:128, 1] = bkf

    # filler kv row: X @ wv.T = -bv exactly, so filler v+bv == 0 on device
    x_fill, *_ = np.linalg.lstsq(wvf, -bvf, rcond=None)  # [D]

    in_maps = []
    per_batch = {}
    for b in range(B):
        mb = mask[b].astype(bool)
        keep = np.flatnonzero(mb)
        cnt = len(keep)
        assert cnt <= NKV, f"unmasked kv count {cnt} exceeds NKV={NKV}"
        xkv_rows = np.empty((NKV, D), np.float32)
        xkv_rows[:cnt] = x[b][keep]
        xkv_rows[cnt:] = x_fill
        xkvT = np.ascontiguousarray(
            xkv_rows.reshape(NKV, DC, 128).transpose(2, 1, 0)
        ).astype(ml_dtypes.bfloat16)                    # [128, DC, NKV]
        maskc = (np.arange(NKV).reshape(SCK, 128).T < cnt).astype(np.float32)
        per_batch[b] = (xkvT, np.ascontiguousarray(maskc))

    for c in range(N_CORES):
        b, half = c // 2, c % 2
        xkvT, maskc = per_batch[b]
        xq = x[b, half * TQ:(half + 1) * TQ]            # [TQ, D]
        xqT = np.ascontiguousarray(
            xq.reshape(QB, 512, DC, 128).transpose(3, 0, 2, 1)
        ).astype(ml_dtypes.bfloat16)                    # [128, QB, DC, 512]
        in_maps.append({
            "xqT": xqT,
            "xkvT": xkvT,
            "wt": wt,
            "bq128": bq128,
            "bkv": bkv,
            "maskc": maskc,
        })
    return in_maps


def run(in_maps, **kwargs):
    nc = _get_nc()
    return run_bass_kernel_spmd(nc, in_maps, core_ids=list(range(N_CORES)), **kwargs)


def kernel(x, mask, wq, bq, wk, bk, wv, bv):
    in_maps = make_in_maps(x, mask, wq, bq, wk, bk, wv, bv)
    res = run(in_maps)
    out = np.empty((B, T, H), dtype=np.float32)
    for c in range(N_CORES):
        b, half = c // 2, c % 2
        o = res.results[c]["out"]                       # [128, TQ//128, H]
        out[b, half * TQ:(half + 1) * TQ] = (
            o.transpose(1, 0, 2).reshape(TQ, H))
    return out
